# revision 4
# baseline (speedup 1.0000x reference)
"""12-bit ripple-carry adder (batch 4194304 x 12, {0,1} floats) on 8 TRN2 NeuronCores.

Algorithm: instead of a 12-step boolean scan, note the outputs are exactly the
bits of S = intA + intB. A single DVE tensor_tensor_scan in Horner form
    state = (u[t] + state) * const[t],   u = A + B (computed by accum-DMA)
with const = 2.0 at the 12 bit positions and 0.0 at a 13th pad position yields
2*S per row (the *0 pad annihilates the running state between rows). Bits are
then extracted as (int(2S) >> (k+1)) & 1 via tensor_scalar.

Sharding: pure data parallel, batch split evenly over the 8 cores.
"""

import numpy as np

import concourse.bass as bass
import concourse.tile as tile
from concourse import bacc, mybir
from concourse.bass_utils import run_bass_kernel_spmd

BATCH = 4194304
BITS = 12
NCORES = 8
ROWS_PC = BATCH // NCORES      # 524288 rows per core
P = 128                        # SBUF partitions
N = 256                        # rows per partition per tile
T = ROWS_PC // (P * N)         # 16 tiles per core
W = BITS + 1                   # padded row width (12 bits + 1 pad/carry slot)

F32 = mybir.dt.float32
I32 = mybir.dt.int32

# extraction j=0..11 -> sum bit (MSB first) = bit (11-j) of S = bit (12-j) of 2S
# j=12 -> carry-out = bit 12 of S = bit 13 of 2S
_SHIFTS = [12 - j for j in range(BITS)] + [13]

# which extraction ops run on gpsimd (rest on DVE) — load balancing knob
GPSIMD_EXTRACT = ()


def _build():
    nc = bacc.Bacc("TRN2", target_bir_lowering=False, debug=False,
                   num_devices=NCORES)
    a_ext = nc.dram_tensor("A", [T, P, N * BITS], F32, kind="ExternalInput").ap()
    b_ext = nc.dram_tensor("B", [T, P, N * BITS], F32, kind="ExternalInput").ap()
    c_ext = nc.dram_tensor("CONST", [P, N * W], F32, kind="ExternalInput").ap()
    o_ext = nc.dram_tensor("out", [T, P, N * W], F32, kind="ExternalOutput").ap()

    with tile.TileContext(nc) as tc:
        with (
            tc.tile_pool(name="const", bufs=1) as cpool,
            tc.tile_pool(name="ab", bufs=2) as abpool,
            tc.tile_pool(name="u", bufs=2) as upool,
            tc.tile_pool(name="scan", bufs=2) as spool,
            tc.tile_pool(name="s2i", bufs=2) as ipool,
            tc.tile_pool(name="oi", bufs=2) as oipool,
            tc.tile_pool(name="out", bufs=2) as opool,
        ):
            const = cpool.tile([P, N * W], F32)
            nc.sync.dma_start(out=const[:], in_=c_ext)

            for t in range(T):
                a = abpool.tile([P, N * BITS], F32, tag="a")
                b = abpool.tile([P, N * BITS], F32, tag="b")
                nc.sync.dma_start(out=a[:], in_=a_ext[t])
                nc.sync.dma_start(out=b[:], in_=b_ext[t])

                u = upool.tile([P, N, W], F32)
                nc.gpsimd.memset(u[:, :, BITS], 0.0)
                nc.gpsimd.tensor_tensor(
                    out=u[:, :, 0:BITS],
                    in0=a[:].rearrange("p (n w) -> p n w", w=BITS),
                    in1=b[:].rearrange("p (n w) -> p n w", w=BITS),
                    op=mybir.AluOpType.add)

                scan = spool.tile([P, N * W], F32)
                nc.vector.tensor_tensor_scan(
                    out=scan[:], data0=u[:].rearrange("p n w -> p (n w)"),
                    data1=const[:], initial=0.0,
                    op0=mybir.AluOpType.add, op1=mybir.AluOpType.mult)

                s2i = ipool.tile([P, N], I32)
                scan3 = scan[:].rearrange("p (n w) -> p n w", w=W)
                nc.scalar.copy(s2i[:], scan3[:, :, BITS - 1])

                oi = oipool.tile([P, N, W], I32)
                for j, sh in enumerate(_SHIFTS):
                    eng = nc.gpsimd if j in GPSIMD_EXTRACT else nc.vector
                    eng.tensor_scalar(
                        out=oi[:, :, j], in0=s2i[:], scalar1=sh, scalar2=1,
                        op0=mybir.AluOpType.logical_shift_right,
                        op1=mybir.AluOpType.bitwise_and)

                out = opool.tile([P, N * W], F32)
                nc.scalar.copy(out[:], oi[:].rearrange("p n w -> p (n w)"))
                nc.scalar.dma_start(out=o_ext[t], in_=out[:])
    nc.compile()
    return nc


_NC = None


def _ensure_built():
    global _NC
    if _NC is None:
        _NC = _build()
    return _NC


def _make_in_maps(A, B):
    A8 = np.ascontiguousarray(A, np.float32).reshape(NCORES, T, P, N * BITS)
    B8 = np.ascontiguousarray(B, np.float32).reshape(NCORES, T, P, N * BITS)
    const = np.tile(np.array([2.0] * BITS + [0.0], np.float32), (P, N))
    return [{"A": A8[i], "B": B8[i], "CONST": const} for i in range(NCORES)]


def _assemble(results):
    full = np.concatenate(
        [results[i]["out"].reshape(ROWS_PC, W) for i in range(NCORES)], axis=0)
    sums = np.ascontiguousarray(full[:, :BITS], dtype=np.float32)
    carry = np.ascontiguousarray(full[:, BITS:BITS + 1], dtype=np.float32)
    return sums, carry


def kernel(A, B):
    nc = _ensure_built()
    res = run_bass_kernel_spmd(nc, _make_in_maps(A, B),
                               core_ids=list(range(NCORES)))
    return _assemble(res.results)
